# revision 1
# baseline (speedup 1.0000x reference)
"""Bass/Trainium2 kernel for nn_BiCRFModel: 2-layer BiLSTM + dense + CRF NLL.

Strategy (8-core pure data parallelism, 32 sequences/core):
  - Gate-input projections (x @ Wx + b) computed as big pre-GEMMs into HBM
    (time-major row layout), streamed back per step.
  - LSTM recurrence in "orientation A": batch(+both directions) in the
    partition dim (64 rows), gates in the free dim.  Recurrent matmuls use
    the transposed hidden state hT as the stationary operand, f32r dtype.
  - Backward direction = global time flip + per-step state masking
    (c,h *= [t < len]), which reproduces tf.reverse_sequence semantics
    exactly without any per-sequence gather.
  - Per-step PE transposes maintain hT and build the transposed layer
    output X{1,2}T in HBM for the next layer's pre-GEMM / dense layer.
  - CRF forward recurrence via a [32, 81] logsumexp (alpha_i + trans_ij),
    per-step validity masking; gold path scores via host-built one-hot /
    pair-count tensors contracted on device against logits / trans.
Output: per-core sum of NLL over its 32 sequences; host sums and /256.
"""

import contextlib

import numpy as np

B, T, E, H, K = 256, 256, 300, 256, 9
N_CORES = 8

_CACHE = {}


# ---------------------------------------------------------------- wait split
def _split_excess_waits(nc, max_waits=1):
    """This walrus build allows only 1 sync wait per instruction.  Hoist
    excess waits onto InstEventSemaphore carriers inserted just before the
    instruction (same engine -> same program order -> identical blocking)."""
    import bass_rust
    import concourse.mybir as mybir

    n_split = 0
    for fn in nc.m.functions:
        for bb in fn.blocks:
            insts = list(bb.instructions)
            out = []
            changed = False
            for ins in insts:
                si = getattr(ins, "sync_info", None)
                waits = list(si.on_wait) if si is not None and si.on_wait else []
                if len(waits) > max_waits:
                    keep = waits[:max_waits]
                    rest = waits[max_waits:]
                    for ci in range(0, len(rest), max_waits):
                        nop = mybir.InstEventSemaphore(
                            name=f"{ins.name}-waitsplit-{ci}", ins=[], outs=[]
                        )
                        nop.engine = ins.engine
                        nop.bass_nofuse = True
                        nop.sync_info = bass_rust.SyncInfo(
                            on_wait=list(rest[ci : ci + max_waits]), on_update=[]
                        )
                        out.append(nop)
                    si.on_wait = keep
                    n_split += 1
                    changed = True
                out.append(ins)
            if changed:
                bb.instructions[:] = out
    return n_split


# ---------------------------------------------------------------- builder
def build_nc(cfg, split=True):
    import concourse.bass as bass
    import concourse.mybir as mybir
    from concourse import tile

    f32 = mybir.dt.float32
    f32r = mybir.dt.float32r
    AF = mybir.ActivationFunctionType
    OP = mybir.AluOpType
    AX = mybir.AxisListType

    Tn = cfg["T"]
    BL = cfg["BL"]
    En = cfg["E"]
    Hn = cfg["H"]
    Kn = cfg["K"]
    EP = -(-En // 128) * 128          # padded input feat
    G4 = 4 * Hn                        # gate width
    HP = 2 * Hn                        # concat feat
    B2 = 2 * BL                        # fw+bw packed batch
    NKE = EP // 128
    NKH = Hn // 128
    NKX = HP // 128
    ROWS = Tn * BL
    NCH = ROWS // 128                  # row chunks
    TPC = 128 // BL                    # timesteps per chunk
    NB = G4 // 512                     # psum n-slices
    K2 = Kn * Kn

    nc = bass.Bass("TRN2", num_devices=cfg["n_cores"])

    embT = nc.dram_tensor("embT", [EP, ROWS], f32r, kind="ExternalInput")
    m2_d = nc.dram_tensor("m2", [B2, Tn], f32, kind="ExternalInput")
    oh_d = nc.dram_tensor("oh", [ROWS, Kn], f32, kind="ExternalInput")
    c81_d = nc.dram_tensor("c81t", [K2, BL], f32, kind="ExternalInput")
    sel_d = nc.dram_tensor("sel", [128, BL], f32, kind="ExternalInput")
    id_d = nc.dram_tensor("identt", [128, BL], f32, kind="ExternalInput")
    tr81_d = nc.dram_tensor("tr81", [BL, K2], f32, kind="ExternalInput")
    tf_d = nc.dram_tensor("transflat", [K2, 1], f32, kind="ExternalInput")
    dw_d = nc.dram_tensor("dwc", [128, NKX * Kn], f32r, kind="ExternalInput")
    db_d = nc.dram_tensor("db", [1, Kn], f32r, kind="ExternalInput")
    on_d = nc.dram_tensor("ones1", [1, 128], f32r, kind="ExternalInput")
    wx_d, wh_d, bias_d = {}, {}, {}
    for l in (0, 1):
        nk = NKE if l == 0 else NKX
        for d in ("f", "b"):
            wx_d[(l, d)] = nc.dram_tensor(f"wx{l}{d}", [128, nk * G4], f32r, kind="ExternalInput")
            bias_d[(l, d)] = nc.dram_tensor(f"bias{l}{d}", [1, G4], f32r, kind="ExternalInput")
        wh_d[l] = nc.dram_tensor(f"wh{l}", [128, 2 * NKH * G4], f32r, kind="ExternalInput")
    out_d = nc.dram_tensor("out", [1, 1], f32, kind="ExternalOutput")

    with tile.TileContext(nc) as tc, contextlib.ExitStack() as ctx:
        cp = ctx.enter_context(tc.tile_pool(name="const", bufs=1))
        gp = ctx.enter_context(tc.tile_pool(name="work", bufs=2))
        sp = ctx.enter_context(tc.tile_pool(name="step", bufs=2))
        pp = ctx.enter_context(tc.tile_pool(name="psum", bufs=2, space="PSUM"))
        dp = ctx.enter_context(tc.tile_pool(name="dram", bufs=1, space="DRAM"))

        def cload(name, dram, shape, dt=f32):
            t = cp.tile(shape, dt, name=name, tag=name)
            nc.sync.dma_start(t[:], dram[:, :])
            return t

        m2s = cload("m2s", m2_d, [B2, Tn])
        c81s = cload("c81s", c81_d, [K2, BL])
        sels = cload("sels", sel_d, [128, BL])
        ids = cload("ids", id_d, [128, BL])
        tr81s = cload("tr81s", tr81_d, [BL, K2])
        tfs = cload("tfs", tf_d, [K2, 1])
        wxs, whs, bss = {}, {}, {}
        for l in (0, 1):
            nk = NKE if l == 0 else NKX
            for d in ("f", "b"):
                wxs[(l, d)] = cload(f"wxs{l}{d}", wx_d[(l, d)], [128, nk * G4], f32r)
                bss[(l, d)] = cload(f"bss{l}{d}", bias_d[(l, d)], [1, G4], f32r)
            whs[l] = cload(f"whs{l}", wh_d[l], [128, 2 * NKH * G4], f32r)
        ones1 = cload("ones1s", on_d, [1, 128], f32r)
        onesb = cp.tile([BL, 1], f32, name="onesb", tag="onesb")
        nc.vector.memset(onesb[:], 1.0)

        xwf = dp.tile([ROWS, G4], f32, name="xwf", tag="xwf")
        xwb = dp.tile([ROWS, G4], f32, name="xwb", tag="xwb")
        x1t = dp.tile([HP, ROWS], f32r, name="x1t", tag="x1t")
        x2t = dp.tile([HP, ROWS], f32r, name="x2t", tag="x2t")
        lgd = dp.tile([ROWS, Kn], f32, name="lgd", tag="lgd")

        # ---------------- pre-GEMM: xw{f,b} = srcT.T @ Wx + b  (rows (t,b))
        def pre_gemm(l, src, nk):
            for ci in range(NCH):
                for d, xwdst in (("f", xwf), ("b", xwb)):
                    c = ci if d == "f" else NCH - 1 - ci
                    xts = []
                    for k in range(nk):
                        xt = gp.tile([128, 128], f32r, name=f"pgx{l}{d}{ci}{k}", tag="pgx", bufs=10)
                        nc.sync.dma_start(
                            xt[:], src[128 * k : 128 * (k + 1), 128 * c : 128 * (c + 1)]
                        )
                        xts.append(xt)
                    zp = pp.tile([128, G4], f32, name=f"pgz{l}{d}{ci}", tag="zp")
                    for n in range(NB):
                        n0 = 512 * n
                        nc.tensor.matmul(
                            zp[:, n0 : n0 + 512],
                            ones1[:],
                            bss[(l, d)][:, n0 : n0 + 512],
                            start=True,
                            stop=False,
                        )
                        for k in range(nk):
                            nc.tensor.matmul(
                                zp[:, n0 : n0 + 512],
                                xts[k][:],
                                wxs[(l, d)][:, k * G4 + n0 : k * G4 + n0 + 512],
                                start=False,
                                stop=(k == nk - 1),
                            )
                    zs = gp.tile([128, G4], f32, name=f"pgs{l}{d}{ci}", tag="pgzs", bufs=3)
                    if ci % 2 == 0:
                        nc.scalar.copy(zs[:], zp[:])
                    else:
                        nc.vector.tensor_copy(zs[:], zp[:])
                    nc.sync.dma_start(xwdst[128 * c : 128 * (c + 1), :], zs[:])

        # ---------------- LSTM step loop (both dirs packed in 64 partitions)
        # Recurrent matmul uses a block-diagonal stationary operand so both
        # directions land in one M=64 base-0 PSUM write (f32r cannot write
        # PSUM at a partition offset): lhsT k-chunks 0..NKH-1 hold hT_fw in
        # cols 0:BL (rest zero), chunks NKH..2NKH-1 hold hT_bw in cols BL:2BL.
        NKH2 = 2 * NKH
        def step_loop(l, xout):
            z0 = sp.tile([128, NKH2 * B2], f32, name=f"z0_{l}", tag="z0")
            nc.vector.memset(z0[:], 0.0)
            hTbig = sp.tile([128, NKH2 * B2], f32r, name=f"hTbig{l}", tag="hTbig", bufs=1)
            nc.scalar.copy(hTbig[:], z0[:])
            c_prev = sp.tile([B2, Hn], f32, name=f"cinit{l}", tag="cst", bufs=3)
            nc.vector.memset(c_prev[:], 0.0)

            def hT_dst(di):
                # strided view: cols di*BL : di*BL+BL of chunks di*NKH..(di+1)*NKH
                return hTbig[:, di * NKH * B2 : (di + 1) * NKH * B2].rearrange(
                    "p (c x) -> p c x", c=NKH
                )[:, :, di * BL : (di + 1) * BL]

            for s in range(min(Tn, cfg.get("nsteps", Tn))):
                tfw, tbw = s, Tn - 1 - s
                xwt = sp.tile([B2, G4], f32, name=f"xw{l}_{s}", tag="xw", bufs=4)
                nc.sync.dma_start(xwt[0:BL, :], xwf[BL * tfw : BL * (tfw + 1), :])
                nc.sync.dma_start(xwt[BL:B2, :], xwb[BL * tbw : BL * (tbw + 1), :])

                zp = pp.tile([B2, G4], f32, name=f"slz{l}_{s}", tag="zp")
                for n in range(NB):
                    n0 = 512 * n
                    for k in range(NKH2):
                        nc.tensor.matmul(
                            zp[:, n0 : n0 + 512],
                            hTbig[:, B2 * k : B2 * (k + 1)],
                            whs[l][:, k * G4 + n0 : k * G4 + n0 + 512],
                            start=(k == 0),
                            stop=(k == NKH2 - 1),
                        )
                zs = sp.tile([B2, G4], f32, name=f"zs{l}_{s}", tag="zs")
                nc.vector.tensor_tensor(zs[:], zp[:], xwt[:], op=OP.add)

                sig = sp.tile([B2, 3 * Hn], f32, name=f"sig{l}_{s}", tag="sig")
                nc.scalar.activation(sig[:], zs[:, 0 : 3 * Hn], AF.Sigmoid)
                g = sp.tile([B2, Hn], f32, name=f"g{l}_{s}", tag="g")
                nc.scalar.activation(g[:], zs[:, 3 * Hn : 4 * Hn], AF.Tanh)

                mcol = m2s[:, s : s + 1]
                t1 = sp.tile([B2, Hn], f32, name=f"t1{l}_{s}", tag="t1")
                nc.vector.scalar_tensor_tensor(t1[:], sig[:, 0:Hn], mcol, g[:], OP.mult, OP.mult)
                t2 = sp.tile([B2, Hn], f32, name=f"t2{l}_{s}", tag="t2")
                nc.vector.scalar_tensor_tensor(
                    t2[:], sig[:, Hn : 2 * Hn], mcol, c_prev[:], OP.mult, OP.mult
                )
                c_new = sp.tile([B2, Hn], f32, name=f"c{l}_{s}", tag="cst", bufs=3)
                nc.vector.tensor_tensor(c_new[:], t1[:], t2[:], op=OP.add)
                th = sp.tile([B2, Hn], f32, name=f"th{l}_{s}", tag="th")
                nc.scalar.activation(th[:], c_new[:], AF.Tanh)
                h = sp.tile([B2, Hn], f32, name=f"h{l}_{s}", tag="h")
                nc.vector.scalar_tensor_tensor(
                    h[:], sig[:, 2 * Hn : 3 * Hn], mcol, th[:], OP.mult, OP.mult
                )

                tps = []
                for di in range(2):
                    tp = pp.tile(
                        [128, NKH * BL], f32, name=f"tp{di}_{l}_{s}",
                        tag=f"tp{di}", bufs=1,
                    )
                    tps.append(tp)
                    po = BL * di
                    for k in range(NKH):
                        nc.tensor.matmul(
                            tp[:, 32 * k : 32 * k + 32],
                            h[po : po + BL, 128 * k : 128 * (k + 1)],
                            ids[po : po + BL, 0:BL],
                            is_transpose=True,
                        )
                for di, d in enumerate(("f", "b")):
                    nc.scalar.copy(
                        hT_dst(di),
                        tps[di][:, :].rearrange("p (c x) -> p c x", c=NKH),
                    )
                    tdst = tfw if d == "f" else tbw
                    for k in range(NKH):
                        nc.sync.dma_start(
                            xout[
                                Hn * di + 128 * k : Hn * di + 128 * (k + 1),
                                BL * tdst : BL * (tdst + 1),
                            ],
                            hTbig[:, (di * NKH + k) * B2 + di * BL : (di * NKH + k) * B2 + di * BL + BL],
                        )
                c_prev = c_new

        def logits_and_crf():
            # ---------------- logits (64 chunks of [128, K]) + unary accumulation
            lg = []
            dws32 = cp.tile([128, NKX * Kn], f32, name="dws32", tag="dws32")
            nc.sync.dma_start(dws32[:], dw_d[:, :].bitcast(f32))
            dbs32 = cp.tile([1, Kn], f32, name="dbs32", tag="dbs32")
            nc.sync.dma_start(dbs32[:], db_d[:, :].bitcast(f32))
            on32 = cp.tile([1, 128], f32, name="on32", tag="on32")
            nc.sync.dma_start(on32[:], on_d[:, :].bitcast(f32))
            usum = cp.tile([128, NCH], f32, name="usum", tag="usum")
            for c in range(NCH):
                lp = pp.tile([128, Kn], f32, name=f"lp{c}", tag="psmall")
                nc.tensor.matmul(lp[:], on32[:], dbs32[:], start=True, stop=False)
                for k in range(NKX):
                    xt = gp.tile([128, 128], f32, name=f"lgx{c}{k}", tag="lgx", bufs=10)
                    nc.sync.dma_start(
                        xt[:],
                        x2t[128 * k : 128 * (k + 1), 128 * c : 128 * (c + 1)].bitcast(f32),
                    )
                    nc.tensor.matmul(
                        lp[:],
                        xt[:],
                        dws32[:, Kn * k : Kn * (k + 1)],
                        start=False,
                        stop=(k == NKX - 1),
                    )
                lgc = cp.tile([128, Kn], f32, name=f"lg{c}", tag=f"lg{c}")
                nc.vector.tensor_copy(lgc[:], lp[:])
                lg.append(lgc)
                nc.sync.dma_start(lgd[128 * c : 128 * (c + 1), :], lgc[:])
                oht = gp.tile([128, Kn], f32, name=f"oht{c}", tag="oht")
                nc.sync.dma_start(oht[:], oh_d[128 * c : 128 * (c + 1), :])
                scr = gp.tile([128, Kn], f32, name=f"ohscr{c}", tag="ohscr")
                nc.vector.scalar_tensor_tensor(
                    scr[:], lgc[:], 1.0, oht[:], OP.mult, OP.mult,
                    accum_out=usum[:, c : c + 1],
                )

            # ---------------- gold-path scores
            up = pp.tile([BL, NCH], f32, name="up", tag="psmall")
            nc.tensor.matmul(up[:], sels[:], usum[:], start=True, stop=True)
            unary = cp.tile([BL, 1], f32, name="unary", tag="unary")
            nc.vector.reduce_sum(unary[:], up[:], axis=AX.X)
            bp = pp.tile([BL, 1], f32, name="bp", tag="psmall")
            nc.tensor.matmul(bp[:], c81s[:], tfs[:], start=True, stop=True)
            binry = cp.tile([BL, 1], f32, name="binry", tag="binry")
            nc.scalar.copy(binry[:], bp[:])

            # ---------------- CRF forward recurrence
            crf = ctx.enter_context(tc.tile_pool(name="crf", bufs=2))
            lgall = cp.tile([BL, Tn * Kn], f32, name="lgall", tag="lgall")
            nc.sync.dma_start(
                lgall[:].rearrange("b (t k) -> b t k", k=Kn),
                lgd[:, :].rearrange("(t b) k -> b t k", b=BL),
            )
            alpha = crf.tile([BL, Kn], f32, name="alpha0", tag="alpha")
            nc.vector.tensor_copy(alpha[:], lgall[:, 0:Kn])
            for t in range(1, Tn):
                mx = crf.tile([BL, 1], f32, name=f"mx{t}", tag="mx")
                nc.vector.reduce_max(mx[:], alpha[:], axis=AX.X)
                nmx = crf.tile([BL, 1], f32, name=f"nmx{t}", tag="nmx")
                nc.vector.tensor_scalar_mul(nmx[:], mx[:], -1.0)
                a81 = crf.tile([BL, K2], f32, name=f"a81_{t}", tag="a81")
                nc.vector.tensor_tensor(
                    a81[:].rearrange("p (j i) -> p j i", i=Kn),
                    alpha[:].unsqueeze(1).broadcast_to([BL, Kn, Kn]),
                    tr81s[:].rearrange("p (j i) -> p j i", i=Kn),
                    op=OP.add,
                )
                e81 = crf.tile([BL, K2], f32, name=f"e81_{t}", tag="e81")
                nc.scalar.activation(e81[:], a81[:], AF.Exp, bias=nmx[:, 0:1])
                s9 = crf.tile([BL, Kn], f32, name=f"s9_{t}", tag="s9")
                nc.vector.reduce_sum(
                    s9[:], e81[:].rearrange("p (j i) -> p j i", i=Kn), axis=AX.X
                )
                lgs = crf.tile([BL, Kn], f32, name=f"lgs{t}", tag="lgs")
                nc.scalar.activation(lgs[:], s9[:], AF.Ln)
                cand = crf.tile([BL, Kn], f32, name=f"cand{t}", tag="cand")
                nc.vector.scalar_tensor_tensor(
                    cand[:], lgs[:], nmx[:, 0:1], lgall[:, Kn * t : Kn * (t + 1)],
                    OP.subtract, OP.add,
                )
                dd = crf.tile([BL, Kn], f32, name=f"dd{t}", tag="dd")
                nc.vector.tensor_tensor(dd[:], cand[:], alpha[:], op=OP.subtract)
                anew = crf.tile([BL, Kn], f32, name=f"alpha{t}", tag="alpha")
                nc.vector.scalar_tensor_tensor(
                    anew[:], dd[:], m2s[0:BL, t : t + 1], alpha[:], OP.mult, OP.add
                )
                alpha = anew

            # ---------------- logZ, nll, partial sum
            mxf = crf.tile([BL, 1], f32, name="mxf", tag="mx")
            nc.vector.reduce_max(mxf[:], alpha[:], axis=AX.X)
            nmxf = crf.tile([BL, 1], f32, name="nmxf", tag="nmx")
            nc.vector.tensor_scalar_mul(nmxf[:], mxf[:], -1.0)
            ef = crf.tile([BL, Kn], f32, name="ef", tag="ef")
            se = crf.tile([BL, 1], f32, name="se", tag="se")
            nc.scalar.activation(ef[:], alpha[:], AF.Exp, bias=nmxf[:, 0:1], accum_out=se[:])
            lgz = crf.tile([BL, 1], f32, name="lgz", tag="lgz")
            nc.scalar.activation(lgz[:], se[:], AF.Ln)
            za = crf.tile([BL, 1], f32, name="za", tag="za")
            nc.vector.tensor_tensor(za[:], lgz[:], nmxf[:], op=OP.subtract)  # logZ
            zb = crf.tile([BL, 1], f32, name="zb", tag="zb")
            nc.vector.tensor_tensor(zb[:], za[:], unary[:], op=OP.subtract)
            nll = crf.tile([BL, 1], f32, name="nll", tag="nll")
            nc.vector.tensor_tensor(nll[:], zb[:], binry[:], op=OP.subtract)
            pf = pp.tile([1, 1], f32, name="pf", tag="psmall")
            nc.tensor.matmul(pf[:], nll[:], onesb[:], start=True, stop=True)
            osb = crf.tile([1, 1], f32, name="osb", tag="osb")
            nc.scalar.copy(osb[:], pf[:])
            nc.sync.dma_start(out_d[:, :], osb[:])

        PH = cfg.get("phase", 99)

        def probe(src_ap):
            pt = cp.tile([1, 1], f32, name="probe", tag="probe")
            nc.sync.dma_start(pt[:], src_ap)
            nc.sync.dma_start(out_d[:, :], pt[:])

        pre_gemm(0, embT, NKE)
        if PH == 1:
            probe(xwf[0:1, 0:1])
        if PH >= 2:
            step_loop(0, x1t)
            if PH == 2:
                probe(x1t[0:1, 0:1].bitcast(f32))
        if PH >= 3:
            pre_gemm(1, x1t, NKX)
            if PH == 3:
                probe(xwf[0:1, 0:1])
        if PH >= 4:
            step_loop(1, x2t)
            if PH == 4:
                probe(x2t[0:1, 0:1].bitcast(f32))
        if PH >= 5:
            logits_and_crf()

    if split:
        _split_excess_waits(nc)
    return nc




# ---------------------------------------------------------------- host prep
def _prep_core(emb_c, lens_c, tgt_c, weights, cfg):
    Tn, BL, En, Hn, Kn = cfg["T"], cfg["BL"], cfg["E"], cfg["H"], cfg["K"]
    EP = -(-En // 128) * 128
    G4 = 4 * Hn
    HP = 2 * Hn
    NKH = Hn // 128
    NKX = HP // 128
    ROWS = Tn * BL
    K2 = Kn * Kn

    perm = np.concatenate(
        [np.arange(0, Hn), np.arange(2 * Hn, 3 * Hn),
         np.arange(3 * Hn, 4 * Hn), np.arange(Hn, 2 * Hn)]
    )

    def prep_wb(w, b):
        wp = np.ascontiguousarray(w[:, perm], np.float32)
        bp = b[perm].astype(np.float32).copy()
        bp[Hn : 2 * Hn] += 1.0
        return wp, bp

    def chunk_k(w, kpad):
        out = np.zeros((kpad, w.shape[1]), np.float32)
        out[: w.shape[0]] = w
        nk = kpad // 128
        return np.ascontiguousarray(
            out.reshape(nk, 128, w.shape[1]).transpose(1, 0, 2).reshape(128, -1)
        )

    d = {}
    et = emb_c.transpose(2, 1, 0).reshape(En, ROWS)
    embT = np.zeros((EP, ROWS), np.float32)
    embT[:En] = et
    d["embT"] = embT

    tt = np.arange(Tn)
    m_fw = (tt[None, :] < lens_c[:, None]).astype(np.float32)
    m_bw = ((Tn - 1 - tt)[None, :] < lens_c[:, None]).astype(np.float32)
    d["m2"] = np.concatenate([m_fw, m_bw], axis=0)

    ohm = np.zeros((ROWS, Kn), np.float32)
    r = tt[:, None] * BL + np.arange(BL)[None, :]          # [T, BL] row ids
    ohm[r.ravel(), tgt_c.T.ravel()] = (tt[:, None] < lens_c[None, :]).astype(
        np.float32
    ).ravel()
    d["oh"] = ohm

    c81 = np.zeros((K2, BL), np.float32)
    for b in range(BL):
        L = int(lens_c[b])
        for t in range(L - 1):
            c81[tgt_c[b, t] * Kn + tgt_c[b, t + 1], b] += 1.0
    d["c81t"] = c81

    d["sel"] = (np.arange(128)[:, None] % BL == np.arange(BL)[None, :]).astype(np.float32)
    d["identt"] = np.tile(np.eye(BL, dtype=np.float32), (128 // BL, 1))
    trans = weights["trans"]
    d["tr81"] = np.tile(trans.T.reshape(1, K2), (BL, 1)).astype(np.float32)
    d["transflat"] = trans.reshape(K2, 1).astype(np.float32)
    dwp = chunk_k(weights["dense_w"].astype(np.float32), HP)
    d["dwc"] = dwp
    d["db"] = weights["dense_b"].reshape(1, Kn).astype(np.float32)
    d["ones1"] = np.ones((1, 128), np.float32)

    for l, (wfk, bfk, wbk, bbk, kin) in enumerate(
        (("w_fw0", "b_fw0", "w_bw0", "b_bw0", EP), ("w_fw1", "b_fw1", "w_bw1", "b_bw1", HP))
    ):
        wh_parts = []
        for dd, (wk, bk) in (("f", (wfk, bfk)), ("b", (wbk, bbk))):
            w, b = prep_wb(weights[wk], weights[bk])
            wx_part = w[: w.shape[0] - Hn]      # input rows
            wh_parts.append(w[w.shape[0] - Hn :])  # recurrent rows (last H)
            d[f"wx{l}{dd}"] = chunk_k(wx_part, kin)
            d[f"bias{l}{dd}"] = b.reshape(1, G4)
        d[f"wh{l}"] = np.concatenate(
            [chunk_k(p, Hn) for p in wh_parts], axis=1
        )
    return d


def _get_runner(cfg):
    key = ("runner", cfg["T"], cfg["BL"], cfg["n_cores"])
    if key in _CACHE:
        return _CACHE[key]
    nc = build_nc(cfg)
    from concourse import bass2jax

    n_cores = cfg["n_cores"]

    import jax
    import numpy as _np
    from jax.sharding import Mesh, PartitionSpec
    from jax.experimental.shard_map import shard_map

    bass2jax.install_neuronx_cc_hook()
    partition_name = nc.partition_id_tensor.name if nc.partition_id_tensor else None
    import concourse.mybir as mybir

    in_names, out_names, out_avals, zero_shapes = [], [], [], []
    for alloc in nc.m.functions[0].allocations:
        if not isinstance(alloc, mybir.MemoryLocationSet):
            continue
        name = alloc.memorylocations[0].name
        if alloc.kind == "ExternalInput":
            if name != partition_name:
                in_names.append(name)
        elif alloc.kind == "ExternalOutput":
            out_names.append(name)
            out_avals.append(
                jax.core.ShapedArray(tuple(alloc.tensor_shape), mybir.dt.np(alloc.dtype))
            )
    n_params = len(in_names)
    all_names = in_names + out_names
    if partition_name is not None:
        all_names = all_names + [partition_name]
    donate = tuple(range(n_params, n_params + len(out_names)))

    def _body(*args):
        operands = list(args)
        if partition_name is not None:
            operands.append(bass2jax.partition_id_tensor())
        outs = bass2jax._bass_exec_p.bind(
            *operands,
            out_avals=tuple(out_avals),
            in_names=tuple(all_names),
            out_names=tuple(out_names),
            lowering_input_output_aliases=(),
            sim_require_finite=True,
            sim_require_nnan=True,
            nc=nc,
        )
        return tuple(outs)

    devices = jax.devices()[:n_cores]

    class Runner:
        pass

    r = Runner()
    r.in_names, r.out_names, r.out_avals, r.n_cores = in_names, out_names, out_avals, n_cores
    if n_cores == 1:
        fn = jax.jit(_body, donate_argnums=donate, keep_unused=True)

        def pack(in_maps):
            return [np.asarray(in_maps[0][n]) for n in in_names]

        def call(packed):
            zeros = [np.zeros(a.shape, a.dtype) for a in out_avals]
            outs = fn(*packed, *zeros)
            return [{n: np.asarray(outs[i]) for i, n in enumerate(out_names)}]
    else:
        from jax.sharding import NamedSharding

        mesh = Mesh(_np.asarray(devices), ("core",))
        fn = jax.jit(
            shard_map(
                _body,
                mesh=mesh,
                in_specs=(PartitionSpec("core"),) * (n_params + len(out_names)),
                out_specs=(PartitionSpec("core"),) * len(out_names),
                check_rep=False,
            ),
            donate_argnums=donate,
            keep_unused=True,
        )
        sh = NamedSharding(mesh, PartitionSpec("core"))

        def pack(in_maps):
            concat_in = [
                np.concatenate([np.asarray(m[n]) for m in in_maps], axis=0)
                for n in in_names
            ]
            return [jax.device_put(a, sh) for a in concat_in]

        def call(packed):
            zeros = [
                np.zeros((n_cores * a.shape[0],) + tuple(a.shape[1:]), a.dtype)
                for a in out_avals
            ]
            outs = fn(*packed, *zeros)
            return [
                {
                    n: np.asarray(outs[i]).reshape((n_cores,) + tuple(out_avals[i].shape))[c]
                    for i, n in enumerate(out_names)
                }
                for c in range(n_cores)
            ]

    r.fn = fn
    r.pack = pack
    r.call = call

    def run(in_maps):
        return call(pack(in_maps))

    r.run = run
    _CACHE[key] = r
    return r


def make_in_maps(inputs, cfg):
    n_cores = cfg["n_cores"]
    BL = cfg["BL"]
    weights = {
        k: np.asarray(inputs[k], np.float32)
        for k in (
            "w_fw0", "b_fw0", "w_bw0", "b_bw0",
            "w_fw1", "b_fw1", "w_bw1", "b_bw1",
            "dense_w", "dense_b", "trans",
        )
    }
    emb = np.asarray(inputs["emb"], np.float32)
    lens = np.asarray(inputs["seq_lens"], np.int64)
    tgt = np.asarray(inputs["targets"], np.int64)
    in_maps = []
    for c in range(n_cores):
        sl = slice(c * BL, (c + 1) * BL)
        in_maps.append(_prep_core(emb[sl], lens[sl], tgt[sl], weights, cfg))
    return in_maps


def kernel(**inputs):
    cfg = dict(T=T, BL=B // N_CORES, E=E, H=H, K=K, n_cores=N_CORES)
    in_maps = make_in_maps(inputs, cfg)
    runner = _get_runner(cfg)
    res = runner.run(in_maps)
    total = sum(float(r["out"][0, 0]) for r in res)
    return np.asarray(np.float32(total / B))



# revision 22
# speedup vs baseline: 7.9203x; 7.9203x over previous
"""Bass/Trainium2 kernel for nn_BiCRFModel: 2-layer BiLSTM + dense + CRF NLL.

Strategy (8-core data parallelism, 32 sequences/core), v2 "transposed
orientation" design:
  - Everything LSTM-side lives transposed: states h/c are [128, 2ch, 64]
    (H=256 in the partition dim x 2 chunks; batch 32fw+32bw in the free dim).
  - Gate pre-activations z are computed directly in a single PSUM bank laid
    out [128, 8 g-chunks, 64]: an identity-stationary matmul injects the
    precomputed input projection xw (start=True), then 32 small fp16 matmuls
    (8 g-chunks x 2 k-chunks x 2 dirs, N=32 each) accumulate Wh^T h.  This
    streams 4x fewer PE rows than the row-orientation design and needs no
    per-step transposes.
  - Sequence masking is folded into the pre-GEMM: the input features are
    augmented with a constant row (bias) and a (1-m) row whose weight is
    -1e4 on the i/f/o gates, so invalid steps drive the gates to 0 and
    zero c/h automatically.  No per-step mask ops.
  - Pre-GEMMs (x @ Wx) run in the same transposed orientation, writing
    xwT [128, dir, gchunk, T, 32] fp16 to DRAM, streamed back per step.
  - h is written fp16 into an 8-step staging tile and flushed to x1T/x2T
    with 4 DMAs per 8 steps.
  - CRF forward runs in the scaled-probability domain: 5 DVE ops per step
    (no per-step exp/ln), exp(trans) and exp(logits - log K) precomputed,
    cheap periodic renormalization for range safety.
Output: per-core sum of NLL over its 32 sequences; host sums and /256.
"""

import contextlib

import numpy as np

B, T, E, H, K = 256, 256, 300, 256, 9
N_CORES = 8

_CACHE = {}

NEG = -10000.0


# ---------------------------------------------------------------- wait split
def _split_excess_waits(nc, max_waits=1):
    """This walrus build allows only 1 sync wait per instruction.  Hoist
    excess waits onto InstEventSemaphore carriers inserted just before the
    instruction (same engine -> same program order -> identical blocking)."""
    import bass_rust
    import concourse.mybir as mybir

    n_split = 0
    for fn in nc.m.functions:
        for bb in fn.blocks:
            insts = list(bb.instructions)
            out = []
            changed = False
            for ins in insts:
                si = getattr(ins, "sync_info", None)
                waits = list(si.on_wait) if si is not None and si.on_wait else []
                if len(waits) > max_waits:
                    keep = waits[:max_waits]
                    rest = waits[max_waits:]
                    for ci in range(0, len(rest), max_waits):
                        nop = mybir.InstEventSemaphore(
                            name=f"{ins.name}-waitsplit-{ci}", ins=[], outs=[]
                        )
                        nop.engine = ins.engine
                        nop.bass_nofuse = True
                        nop.sync_info = bass_rust.SyncInfo(
                            on_wait=list(rest[ci : ci + max_waits]), on_update=[]
                        )
                        out.append(nop)
                    si.on_wait = keep
                    n_split += 1
                    changed = True
                out.append(ins)
            if changed:
                bb.instructions[:] = out
    return n_split


# ---------------------------------------------------------------- builder
def build_nc(cfg, split=True):
    import concourse.bass as bass
    import concourse.mybir as mybir
    from concourse import tile

    f32 = mybir.dt.float32
    f16 = mybir.dt.float16
    AF = mybir.ActivationFunctionType
    OP = mybir.AluOpType
    AX = mybir.AxisListType

    Tn = cfg["T"]
    BL = cfg["BL"]          # 32 sequences per core
    Hn = cfg["H"]           # 256
    Kn = cfg["K"]           # 9
    B2 = 2 * BL             # 64: fw cols 0:BL, bw cols BL:B2
    G4 = 4 * Hn             # 1024
    NG = G4 // 128          # 8 gate chunks: i=0,1 f=2,3 o=4,5 j=6,7
    NC = Hn // 128          # 2 h chunks
    HP = 2 * Hn             # 512 concat feature
    ROWS = Tn * BL
    NBLK = ROWS // 512      # pre-GEMM row blocks
    TPB = 512 // BL         # timesteps per block (16)
    K2 = Kn * Kn
    NKX = HP // 128         # 4
    NK0 = 3                 # ceil((E+2)/128): emb + ones + negmask rows
    NK1 = NKX + 1           # x1 chunks + const (ones/negmask) chunk
    PER = 8                 # h staging period
    RENORM = 64

    nc = bass.Bass("TRN2", num_devices=cfg["n_cores"])

    embT = nc.dram_tensor("embT", [128, NK0 * ROWS], f16, kind="ExternalInput")
    x1c = nc.dram_tensor("x1c", [128, ROWS], f16, kind="ExternalInput")
    id_d = nc.dram_tensor("identp", [128, 128], f16, kind="ExternalInput")
    wx_d, wh_d = {}, {}
    for l in (0, 1):
        nk = NK0 if l == 0 else NK1
        wx_d[l] = nc.dram_tensor(f"wx{l}", [128, 2 * nk * NG * 128], f16, kind="ExternalInput")
        wh_d[l] = nc.dram_tensor(f"wh{l}", [128, 2 * NC * NG * 128], f16, kind="ExternalInput")
    m2_d = nc.dram_tensor("m2", [BL, Tn], f32, kind="ExternalInput")
    oh_d = nc.dram_tensor("oh", [ROWS, Kn], f32, kind="ExternalInput")
    c81_d = nc.dram_tensor("c81t", [K2, BL], f32, kind="ExternalInput")
    sel_d = nc.dram_tensor("sel", [128, BL], f32, kind="ExternalInput")
    etr_d = nc.dram_tensor("etr81", [BL, K2], f32, kind="ExternalInput")
    etrb_d = nc.dram_tensor("etr81b", [BL, K2], f32, kind="ExternalInput")
    tf_d = nc.dram_tensor("transflat", [K2, 1], f32, kind="ExternalInput")
    dw_d = nc.dram_tensor("dwc", [128, NKX * Kn], f16, kind="ExternalInput")
    db_d = nc.dram_tensor("db", [1, Kn], f32, kind="ExternalInput")
    on_d = nc.dram_tensor("ones1", [1, 128], f32, kind="ExternalInput")
    out_d = nc.dram_tensor("out", [1, 1], f32, kind="ExternalOutput")

    with tile.TileContext(nc) as tc, contextlib.ExitStack() as ctx:
        cp = ctx.enter_context(tc.tile_pool(name="const", bufs=1))
        gp = ctx.enter_context(tc.tile_pool(name="work", bufs=2))
        sp = ctx.enter_context(tc.tile_pool(name="step", bufs=2))
        pp = ctx.enter_context(tc.tile_pool(name="psum", bufs=2, space="PSUM"))
        dp = ctx.enter_context(tc.tile_pool(name="dram", bufs=1, space="DRAM"))

        def cload(name, dram, shape, dt=f32):
            t = cp.tile(shape, dt, name=name, tag=name)
            nc.sync.dma_start(t[:], dram[:, :])
            return t

        ident = cload("idents", id_d, [128, 128], f16)
        m2s = cload("m2s", m2_d, [BL, Tn])
        c81s = cload("c81s", c81_d, [K2, BL])
        sels = cload("sels", sel_d, [128, BL])
        etr81s = cload("etr81s", etr_d, [BL, K2])
        etr81bs = cload("etr81bs", etrb_d, [BL, K2])
        tfs = cload("tfs", tf_d, [K2, 1])
        x1cs = cload("x1cs", x1c, [128, ROWS], f16)
        wxs, whs = {}, {}
        for l in (0, 1):
            nk = NK0 if l == 0 else NK1
            wxs[l] = cload(f"wxs{l}", wx_d[l], [128, 2 * nk * NG * 128], f16)
            whs[l] = cload(f"whs{l}", wh_d[l], [128, 2 * NC * NG * 128], f16)
        ones1 = cload("ones1s", on_d, [1, 128])
        onesb = cp.tile([BL, 1], f32, name="onesb", tag="onesb")
        nc.vector.memset(onesb[:], 1.0)
        z9 = cp.tile([BL, Kn], f32, name="z9", tag="z9")
        nc.vector.memset(z9[:], 0.0)

        # DRAM intermediates
        xwt = {}
        for l in (0, 1):
            xwt[l] = dp.tile([128, 2, NG, Tn, BL], f16, name=f"xwt{l}", tag=f"xwt{l}")
        x1t = dp.tile([HP, ROWS], f16, name="x1t", tag="x1t")
        x2t = dp.tile([HP, ROWS], f16, name="x2t", tag="x2t")
        lgd = dp.tile([ROWS, Kn], f32, name="lgd", tag="lgd")

        # ---------------- pre-GEMM (transposed): xwT[g, (t,b)] per dir
        # moving = input chunks [128, 512 rows]; stationary = Wx [128k, 128g].
        # Emitted as a list of work-item closures so blocks can be
        # interleaved into the previous layer's step loop.
        def pre_gemm_items(l, mv_load, nk, blk_order=None, copy_eng="av"):
            items = []
            if blk_order is None:
                # interleave low/high blocks: fw consumes low-t first, bw high-t
                blk_order = []
                for i in range((NBLK + 1) // 2):
                    blk_order.append(i)
                    if NBLK - 1 - i != i:
                        blk_order.append(NBLK - 1 - i)
            for blk in blk_order:
                def load_item(blk=blk):
                    return [mv_load(kc, blk) for kc in range(nk)]
                items.append(("load", blk, load_item))
                for d in ((1, 0) if blk >= NBLK // 2 else (0, 1)):
                    for gc in range(NG):
                        def group_item(mvs, l=l, blk=blk, d=d, gc=gc):
                            pg = pp.tile([128, 512], f32,
                                         name=f"pg{l}{blk}{d}{gc}", tag="pg")
                            for kc in range(nk):
                                w0 = ((d * nk + kc) * NG + gc) * 128
                                nc.tensor.matmul(
                                    pg[:],
                                    wxs[l][:, w0 : w0 + 128],
                                    mvs[kc],
                                    start=(kc == 0),
                                    stop=(kc == nk - 1),
                                )
                            zs = gp.tile([128, 512], f16,
                                         name=f"pgs{l}{blk}{d}{gc}",
                                         tag="pgzs", bufs=4)
                            if copy_eng == "gp":
                                nc.gpsimd.tensor_copy(zs[:], pg[:])
                            elif gc % 2 == 0:
                                nc.scalar.copy(zs[:], pg[:])
                            else:
                                nc.vector.tensor_copy(zs[:], pg[:])
                            nc.sync.dma_start(
                                xwt[l][:, d, gc, blk * TPB : (blk + 1) * TPB, :]
                                .rearrange("p t b -> p (t b)"),
                                zs[:],
                            )
                        items.append(("group", blk, group_item))
            return items

        def run_items(items):
            mvs_by_blk = {}
            for kind, blk, fn in items:
                if kind == "load":
                    mvs_by_blk[blk] = fn()
                else:
                    fn(mvs_by_blk[blk])

        def mk_emb_loader():
            def load(kc, blk):
                t = gp.tile([128, 512], f16, name=f"emb{kc}{blk}", tag="mv", bufs=6)
                nc.sync.dma_start(
                    t[:], embT[:, kc * ROWS + blk * 512 : kc * ROWS + (blk + 1) * 512]
                )
                return t[:]
            return load

        def mk_x1_loader():
            def load(kc, blk):
                if kc == NKX:
                    return x1cs[:, blk * 512 : (blk + 1) * 512]
                t = gp.tile([128, 512], f16, name=f"x1m{kc}{blk}", tag="mv", bufs=6)
                nc.sync.dma_start(
                    t[:], x1t[kc * 128 : (kc + 1) * 128, blk * 512 : (blk + 1) * 512]
                )
                return t[:]
            return load

        # ---------------- LSTM step loop, transposed orientation
        # side_work: list of (kind, blk, fn) work items; ready_at maps blk ->
        # first step s after which its inputs exist.  Items are drained in
        # order, a few per step, once ready.
        def step_loop(l, xout, side_work=None, ready_at=None, per_step=3):
            sw = list(side_work) if side_work else []
            sw_pos = 0
            mvs_by_blk = {}

            def drain_side(s, budget):
                nonlocal sw_pos
                n = 0
                while sw_pos < len(sw) and n < budget:
                    kind, blk, fn = sw[sw_pos]
                    if ready_at is not None and ready_at[blk] > s:
                        break
                    if kind == "load":
                        mvs_by_blk[blk] = fn()
                    else:
                        fn(mvs_by_blk[blk])
                    sw_pos += 1
                    n += 1

            hz = sp.tile([128, NC, BL], f16, name=f"hz{l}", tag="hz", bufs=1)
            nc.vector.memset(hz[:], 0.0)
            cz, c_prev, h_prev, stage, xwi, zp = {}, {}, {}, {}, {}, {}
            sig, g, ta, tb, c_new, th = {}, {}, {}, {}, {}, {}
            for d in range(2):
                cz[d] = sp.tile([128, NC, BL], f32, name=f"cz{l}{d}",
                                tag=f"cst{d}", bufs=3)
                nc.vector.memset(cz[d][:], 0.0)
                c_prev[d] = cz[d]
                h_prev[d] = (hz, None)
                stage[d] = None
            nsteps = min(Tn, cfg.get("nsteps", Tn))
            gorder = [6, 7, 0, 1, 2, 3, 4, 5]
            for s in range(nsteps):
                tstep = {0: s, 1: Tn - 1 - s}
                slot = {0: s % PER, 1: PER - 1 - (s % PER)}
                if s % PER == 0:
                    for d in range(2):
                        stage[d] = sp.tile([128, NC, PER, BL], f16,
                                           name=f"st{l}{d}_{s}",
                                           tag=f"stage{d}", bufs=2)
                for d in range(2):
                    xwi[d] = sp.tile([128, NG, BL], f16, name=f"xw{l}{d}_{s}",
                                     tag=f"xw{d}", bufs=6)
                    nc.sync.dma_start(xwi[d][:], xwt[l][:, d, :, tstep[d], :])
                # matmuls: inject xw then accumulate Wh^T h, j chunks first
                for d in range(2):
                    zp[d] = pp.tile([128, NG, 64], f32, name=f"z{l}{d}_{s}",
                                    tag=f"zp{d}", bufs=2)
                    for i_g, gc in enumerate(gorder):
                        nc.tensor.matmul(
                            zp[d][:, gc, 0:BL], ident[:], xwi[d][:, gc, :],
                            start=(i_g == 0), stop=False,
                        )
                    n_mm = 0
                    for gc in gorder:
                        for c in range(NC):
                            n_mm += 1
                            ht, sl = h_prev[d]
                            hp_ap = ht[:, c, :] if sl is None else ht[:, c, sl, :]
                            w0 = ((d * NC + c) * NG + gc) * 128
                            nc.tensor.matmul(
                                zp[d][:, gc, 0:BL],
                                whs[l][:, w0 : w0 + 128],
                                hp_ap,
                                start=False,
                                stop=(n_mm == NC * NG),
                            )
                for d in range(2):
                    g[d] = sp.tile([128, NC, BL], f32, name=f"g{l}{d}_{s}",
                                   tag=f"g{d}")
                    nc.scalar.activation(g[d][:], zp[d][:, 6:8, 0:BL], AF.Tanh)
                for d in range(2):
                    sig[d] = sp.tile([128, 6, BL], f32, name=f"sg{l}{d}_{s}",
                                     tag=f"sg{d}")
                    nc.scalar.activation(sig[d][:], zp[d][:, 0:6, 0:BL], AF.Sigmoid)
                for d in range(2):
                    tb[d] = sp.tile([128, NC, BL], f32, name=f"tb{l}{d}_{s}",
                                    tag=f"tb{d}")
                    nc.gpsimd.tensor_tensor(tb[d][:], sig[d][:, 2:4, :],
                                            c_prev[d][:], op=OP.mult)
                    ta[d] = sp.tile([128, NC, BL], f32, name=f"ta{l}{d}_{s}",
                                    tag=f"ta{d}")
                    nc.vector.tensor_tensor(ta[d][:], sig[d][:, 0:2, :], g[d][:],
                                            op=OP.mult)
                for d in range(2):
                    c_new[d] = sp.tile([128, NC, BL], f32, name=f"c{l}{d}_{s}",
                                       tag=f"cst{d}", bufs=3)
                    nc.vector.tensor_tensor(c_new[d][:], ta[d][:], tb[d][:],
                                            op=OP.add)
                for d in range(2):
                    th[d] = sp.tile([128, NC, BL], f32, name=f"th{l}{d}_{s}",
                                    tag=f"th{d}")
                    nc.scalar.activation(th[d][:], c_new[d][:], AF.Tanh)
                for d in range(2):
                    eng = nc.vector if d == 0 else nc.gpsimd
                    eng.tensor_tensor(
                        stage[d][:, :, slot[d], :], sig[d][:, 4:6, :], th[d][:],
                        op=OP.mult,
                    )
                    h_prev[d] = (stage[d], slot[d])
                    c_prev[d] = c_new[d]

                if s % PER == PER - 1:
                    t0 = {0: s - (PER - 1), 1: Tn - 1 - s}
                    for d in range(2):
                        for c in range(NC):
                            nc.sync.dma_start(
                                xout[d * Hn + c * 128 : d * Hn + (c + 1) * 128,
                                     t0[d] * BL : (t0[d] + PER) * BL],
                                stage[d][:, c, :, :],
                            )
                drain_side(s, per_step)
            drain_side(10 ** 9, 10 ** 9)

        # ---------------- logits + gold-path + CRF
        dws = cload("dws", dw_d, [128, NKX * Kn], f16)
        dbs = cload("dbs", db_d, [1, Kn])
        usum = cp.tile([128, ROWS // 128], f32, name="usum", tag="usum")

        def logits_items():
            items = []
            for cch in range(ROWS // 128):
                def chunk_item(cch=cch):
                    lp = pp.tile([128, Kn], f32, name=f"lp{cch}", tag="psmall")
                    nc.tensor.matmul(lp[:], ones1[:], dbs[:], start=True, stop=False)
                    for kc in range(NKX):
                        xt = gp.tile([128, 128], f16, name=f"lgx{cch}{kc}",
                                     tag="lgx", bufs=8)
                        nc.sync.dma_start(
                            xt[:],
                            x2t[kc * 128 : (kc + 1) * 128,
                                cch * 128 : (cch + 1) * 128],
                        )
                        nc.tensor.matmul(
                            lp[:], xt[:], dws[:, Kn * kc : Kn * (kc + 1)],
                            start=False, stop=(kc == NKX - 1),
                        )
                    lgc = gp.tile([128, Kn], f32, name=f"lg{cch}", tag="lgcp", bufs=4)
                    nc.vector.tensor_copy(lgc[:], lp[:])
                    nc.sync.dma_start(lgd[cch * 128 : (cch + 1) * 128, :], lgc[:])
                    oht = gp.tile([128, Kn], f32, name=f"oht{cch}", tag="oht", bufs=4)
                    nc.sync.dma_start(oht[:], oh_d[cch * 128 : (cch + 1) * 128, :])
                    scr = gp.tile([128, Kn], f32, name=f"ohscr{cch}", tag="ohscr",
                                  bufs=4)
                    nc.vector.scalar_tensor_tensor(
                        scr[:], lgc[:], 1.0, oht[:], OP.mult, OP.mult,
                        accum_out=usum[:, cch : cch + 1],
                    )
                items.append(("load", cch, chunk_item))
            return items

        def crf_tail():
            # gold-path scores
            up = pp.tile([BL, ROWS // 128], f32, name="up", tag="psmall")
            nc.tensor.matmul(up[:], sels[:], usum[:], start=True, stop=True)
            unary = cp.tile([BL, 1], f32, name="unary", tag="unary")
            nc.vector.reduce_sum(unary[:], up[:], axis=AX.X)
            bp = pp.tile([BL, 1], f32, name="bp", tag="psmall")
            nc.tensor.matmul(bp[:], c81s[:], tfs[:], start=True, stop=True)
            binry = cp.tile([BL, 1], f32, name="binry", tag="binry")
            nc.scalar.copy(binry[:], bp[:])

            # CRF forward in scaled-probability domain
            crf = ctx.enter_context(tc.tile_pool(name="crf", bufs=2))
            lgall = cp.tile([BL, Tn * Kn], f32, name="lgall", tag="lgall")
            nc.sync.dma_start(
                lgall[:].rearrange("b (t k) -> b t k", k=Kn),
                lgd[:, :].rearrange("(t b) k -> b t k", b=BL),
            )
            el = cp.tile([BL, Tn * Kn], f32, name="el", tag="el")
            nc.scalar.activation(el[:], lgall[:], AF.Exp)
            MID = cfg.get("crfM", 160)
            p_cur = crf.tile([BL, Kn], f32, name="p0", tag="alpha", bufs=3)
            nc.vector.tensor_copy(p_cur[:], el[:, 0:Kn])
            acc = crf.tile([BL, 1], f32, name="acc", tag="accr", bufs=2)
            nc.vector.memset(acc[:], 0.0)
            b_cur = crf.tile([BL, Kn], f32, name="b0", tag="beta", bufs=3)
            nc.vector.memset(b_cur[:], 1.0)
            accb = crf.tile([BL, 1], f32, name="accb", tag="accbr", bufs=2)
            nc.vector.memset(accb[:], 0.0)

            # alpha: p_t = mask( (p @ Etr) * el_t ), t = 1..MID  (DVE)
            # beta:  B_t = mask( Etr_b @ (B_{t+1} * el_{t+1}) ), t = T-2..MID (gpsimd)
            a_ts = list(range(1, MID + 1))
            b_ts = list(range(Tn - 2, MID - 1, -1))
            for i in range(max(len(a_ts), len(b_ts))):
                if i < len(a_ts):
                    t = a_ts[i]
                    q81 = crf.tile([BL, K2], f32, name=f"q{t}", tag="q81")
                    nc.vector.tensor_tensor(
                        q81[:].rearrange("p (i j) -> p i j", j=Kn),
                        p_cur[:].unsqueeze(1).broadcast_to([BL, Kn, Kn]),
                        etr81s[:].rearrange("p (i j) -> p i j", j=Kn),
                        op=OP.mult,
                    )
                    s9 = crf.tile([BL, Kn], f32, name=f"s9_{t}", tag="s9")
                    nc.vector.reduce_sum(
                        s9[:], q81[:].rearrange("p (i j) -> p i j", j=Kn), axis=AX.X
                    )
                    u9 = crf.tile([BL, Kn], f32, name=f"u{t}", tag="u9")
                    nc.vector.tensor_tensor(
                        u9[:], s9[:], el[:, Kn * t : Kn * (t + 1)], op=OP.mult
                    )
                    dd = crf.tile([BL, Kn], f32, name=f"dd{t}", tag="dd")
                    nc.vector.tensor_tensor(dd[:], u9[:], p_cur[:], op=OP.subtract)
                    p_new = crf.tile([BL, Kn], f32, name=f"p{t}", tag="alpha", bufs=3)
                    nc.vector.scalar_tensor_tensor(
                        p_new[:], dd[:], m2s[:, t : t + 1], p_cur[:], OP.mult, OP.add
                    )
                    p_cur = p_new
                    if t % RENORM == 0:
                        rmax = crf.tile([BL, 1], f32, name=f"rm{t}", tag="rm")
                        nc.vector.reduce_max(rmax[:], p_cur[:], axis=AX.X)
                        rinv = crf.tile([BL, 1], f32, name=f"ri{t}", tag="ri")
                        nc.vector.reciprocal(rinv[:], rmax[:])
                        p_sc = crf.tile([BL, Kn], f32, name=f"ps{t}", tag="alpha",
                                        bufs=3)
                        nc.vector.scalar_tensor_tensor(
                            p_sc[:], p_cur[:], rinv[:, 0:1], z9[:], OP.mult, OP.add
                        )
                        p_cur = p_sc
                        lnr = crf.tile([BL, 1], f32, name=f"lr{t}", tag="lr")
                        nc.scalar.activation(lnr[:], rmax[:], AF.Ln)
                        acc2 = crf.tile([BL, 1], f32, name=f"acc{t}", tag="accr",
                                        bufs=2)
                        nc.vector.tensor_tensor(acc2[:], acc[:], lnr[:], op=OP.add)
                        acc = acc2
                if i < len(b_ts):
                    t = b_ts[i]
                    w9 = crf.tile([BL, Kn], f32, name=f"w{t}", tag="w9")
                    nc.vector.tensor_tensor(
                        w9[:], b_cur[:], el[:, Kn * (t + 1) : Kn * (t + 2)],
                        op=OP.mult,
                    )
                    qb = crf.tile([BL, K2], f32, name=f"qb{t}", tag="qb81")
                    nc.vector.tensor_tensor(
                        qb[:].rearrange("p (i j) -> p i j", j=Kn),
                        w9[:].unsqueeze(1).broadcast_to([BL, Kn, Kn]),
                        etr81bs[:].rearrange("p (i j) -> p i j", j=Kn),
                        op=OP.mult,
                    )
                    s9b = crf.tile([BL, Kn], f32, name=f"s9b{t}", tag="s9b")
                    nc.vector.reduce_sum(
                        s9b[:], qb[:].rearrange("p (i j) -> p i j", j=Kn), axis=AX.X
                    )
                    ddb = crf.tile([BL, Kn], f32, name=f"ddb{t}", tag="ddb")
                    nc.vector.tensor_tensor(ddb[:], s9b[:], b_cur[:],
                                            op=OP.subtract)
                    b_new = crf.tile([BL, Kn], f32, name=f"b{t}", tag="beta", bufs=3)
                    nc.vector.scalar_tensor_tensor(
                        b_new[:], ddb[:], m2s[:, t + 1 : t + 2], b_cur[:],
                        OP.mult, OP.add,
                    )
                    b_cur = b_new
                    if t % RENORM == 0:
                        rmb = crf.tile([BL, 1], f32, name=f"rmb{t}", tag="rmb")
                        nc.vector.reduce_max(rmb[:], b_cur[:], axis=AX.X)
                        rib = crf.tile([BL, 1], f32, name=f"rib{t}", tag="rib")
                        nc.vector.reciprocal(rib[:], rmb[:])
                        b_sc = crf.tile([BL, Kn], f32, name=f"bs{t}", tag="beta",
                                        bufs=3)
                        nc.vector.scalar_tensor_tensor(
                            b_sc[:], b_cur[:], rib[:, 0:1], z9[:], OP.mult, OP.add
                        )
                        b_cur = b_sc
                        lnb = crf.tile([BL, 1], f32, name=f"lnb{t}", tag="lnb")
                        nc.scalar.activation(lnb[:], rmb[:], AF.Ln)
                        accb2 = crf.tile([BL, 1], f32, name=f"accb{t}", tag="accbr",
                                         bufs=2)
                        nc.vector.tensor_tensor(accb2[:], accb[:], lnb[:], op=OP.add)
                        accb = accb2

            # logZ' = ln(sum p_MID * B_MID) + acc + accb
            ub = crf.tile([BL, Kn], f32, name="ub", tag="ub")
            nc.vector.tensor_tensor(ub[:], p_cur[:], b_cur[:], op=OP.mult)
            se = crf.tile([BL, 1], f32, name="se", tag="se")
            nc.vector.reduce_sum(se[:], ub[:], axis=AX.X)
            lgz = crf.tile([BL, 1], f32, name="lgz", tag="lgz")
            nc.scalar.activation(lgz[:], se[:], AF.Ln)
            zaa = crf.tile([BL, 1], f32, name="zaa", tag="zaa")
            nc.vector.tensor_tensor(zaa[:], lgz[:], acc[:], op=OP.add)
            za = crf.tile([BL, 1], f32, name="za", tag="za")
            nc.vector.tensor_tensor(za[:], zaa[:], accb[:], op=OP.add)
            zb = crf.tile([BL, 1], f32, name="zb", tag="zb")
            nc.vector.tensor_tensor(zb[:], za[:], unary[:], op=OP.subtract)
            nll = crf.tile([BL, 1], f32, name="nll", tag="nll")
            nc.vector.tensor_tensor(nll[:], zb[:], binry[:], op=OP.subtract)
            pf = pp.tile([1, 1], f32, name="pf", tag="psmall")
            nc.tensor.matmul(pf[:], nll[:], onesb[:], start=True, stop=True)
            osb = crf.tile([1, 1], f32, name="osb", tag="osb")
            nc.scalar.copy(osb[:], pf[:])
            nc.sync.dma_start(out_d[:, :], osb[:])

        PH = cfg.get("phase", 99)
        ILV = cfg.get("ilv", 1)

        def probe(src_ap):
            pt = cp.tile([1, 1], f32, name="probe", tag="probe")
            nc.sync.dma_start(pt[:], src_ap)
            nc.sync.dma_start(out_d[:, :], pt[:])

        run_items(pre_gemm_items(0, mk_emb_loader(), NK0))
        if PH == 1:
            probe(xwt[0][0:1, 0, 0, 0, 0:2].bitcast(f32))

        def f8(x):
            return ((x + 8) // 8) * 8 - 1

        if PH >= 2:
            if ILV and PH >= 3:
                rb1 = {b: max(TPB * b + TPB - 1, Tn - 1 - TPB * b)
                       for b in range(NBLK)}
                order1 = sorted(range(NBLK), key=lambda b: rb1[b])
                step_loop(0, x1t,
                          side_work=pre_gemm_items(1, mk_x1_loader(), NK1,
                                                   blk_order=order1),
                          ready_at=rb1, per_step=3)
            else:
                step_loop(0, x1t)
        if PH >= 3 and not ILV:
            run_items(pre_gemm_items(1, mk_x1_loader(), NK1))
        if PH >= 4:
            if ILV and PH >= 5:
                rb2 = {c: max(f8(4 * c + 3), f8(Tn - 1 - 4 * c))
                       for c in range(ROWS // 128)}
                order2 = sorted(range(ROWS // 128), key=lambda c: rb2[c])
                items2 = logits_items()
                items2 = [items2[c] for c in order2]
                step_loop(1, x2t, side_work=items2, ready_at=rb2, per_step=1)
            else:
                step_loop(1, x2t)
        if PH >= 5:
            if not ILV:
                run_items(logits_items())
            crf_tail()

    if split:
        _split_excess_waits(nc)
    return nc


# ---------------------------------------------------------------- host prep
def _prep_core(emb_c, lens_c, tgt_c, weights, cfg):
    Tn, BL, En, Hn, Kn = cfg["T"], cfg["BL"], cfg["E"], cfg["H"], cfg["K"]
    G4 = 4 * Hn
    HP = 2 * Hn
    NG = G4 // 128
    NC = Hn // 128
    NKX = HP // 128
    NK0, NK1 = 3, NKX + 1
    ROWS = Tn * BL
    K2 = Kn * Kn
    c0 = float(np.log(Kn))

    perm = np.concatenate(
        [np.arange(0, Hn), np.arange(2 * Hn, 3 * Hn),
         np.arange(3 * Hn, 4 * Hn), np.arange(Hn, 2 * Hn)]
    )

    tt = np.arange(Tn)
    m = (tt[None, :] < lens_c[:, None]).astype(np.float32)      # [BL, T]
    negm_rows = (1.0 - m.T).reshape(ROWS)                        # (t, b) flat

    d = {}
    # augmented embT: rows 0..E-1 emb, row E ones (bias), row E+1 negmask
    et = emb_c.transpose(2, 1, 0).reshape(En, ROWS)
    embT = np.zeros((NK0 * 128, ROWS), np.float32)
    embT[:En] = et
    embT[En] = 1.0
    embT[En + 1] = negm_rows
    d["embT"] = embT.reshape(NK0, 128, ROWS).transpose(1, 0, 2).reshape(
        128, NK0 * ROWS).astype(np.float16)

    # constant 5th chunk for layer 1: row0 ones, row1 negmask
    x1c = np.zeros((128, ROWS), np.float32)
    x1c[0] = 1.0
    x1c[1] = negm_rows
    d["x1c"] = x1c.astype(np.float16)

    d["identp"] = np.eye(128, dtype=np.float16)

    NEG_ROW = np.zeros(G4, np.float32)
    NEG_ROW[0 : 3 * Hn] = NEG                                     # i, f, o gates

    def wx_pack(l, w_fw, b_fw, w_bw, b_bw, nk):
        # build augmented Wx [nk*128, G4] per dir then pack stationary chunks
        out = np.zeros((128, 2 * nk * NG * 128), np.float32)
        for di, (w, b) in enumerate(((w_fw, b_fw), (w_bw, b_bw))):
            wp = w[:, perm].astype(np.float32)
            bp = b[perm].astype(np.float32).copy()
            bp[Hn : 2 * Hn] += 1.0                                # forget bias
            kin = wp.shape[0] - Hn                                # input rows
            wx = np.zeros((nk * 128, G4), np.float32)
            wx[:kin] = wp[:kin]
            if l == 0:
                wx[kin] = bp                                      # ones row
                wx[kin + 1] = NEG_ROW                             # negmask row
            else:
                wx[NKX * 128] = bp                                # const chunk row0
                wx[NKX * 128 + 1] = NEG_ROW
            for kc in range(nk):
                for gc in range(NG):
                    w0 = ((di * nk + kc) * NG + gc) * 128
                    out[:, w0 : w0 + 128] = wx[kc * 128 : (kc + 1) * 128,
                                               gc * 128 : (gc + 1) * 128]
        return out.astype(np.float16)

    def wh_pack(w_fw, w_bw):
        out = np.zeros((128, 2 * NC * NG * 128), np.float32)
        for di, w in enumerate((w_fw, w_bw)):
            wp = w[:, perm].astype(np.float32)
            whp = wp[wp.shape[0] - Hn :]                          # recurrent rows
            for c in range(NC):
                for gc in range(NG):
                    w0 = ((di * NC + c) * NG + gc) * 128
                    out[:, w0 : w0 + 128] = whp[c * 128 : (c + 1) * 128,
                                                gc * 128 : (gc + 1) * 128]
        return out.astype(np.float16)

    d["wx0"] = wx_pack(0, weights["w_fw0"], weights["b_fw0"],
                       weights["w_bw0"], weights["b_bw0"], NK0)
    d["wx1"] = wx_pack(1, weights["w_fw1"], weights["b_fw1"],
                       weights["w_bw1"], weights["b_bw1"], NK1)
    d["wh0"] = wh_pack(weights["w_fw0"], weights["w_bw0"])
    d["wh1"] = wh_pack(weights["w_fw1"], weights["w_bw1"])

    d["m2"] = m

    ohm = np.zeros((ROWS, Kn), np.float32)
    r = tt[:, None] * BL + np.arange(BL)[None, :]
    ohm[r.ravel(), tgt_c.T.ravel()] = (tt[:, None] < lens_c[None, :]).astype(
        np.float32).ravel()
    d["oh"] = ohm

    c81 = np.zeros((K2, BL), np.float32)
    for b in range(BL):
        L = int(lens_c[b])
        for t in range(L - 1):
            c81[tgt_c[b, t] * Kn + tgt_c[b, t + 1], b] += 1.0
    d["c81t"] = c81

    d["sel"] = (np.arange(128)[:, None] % BL == np.arange(BL)[None, :]).astype(np.float32)
    trans = weights["trans"].astype(np.float32)
    d["etr81"] = np.tile(np.exp(trans).T.reshape(1, K2), (BL, 1)).astype(np.float32)
    d["etr81b"] = np.tile(np.exp(trans).reshape(1, K2), (BL, 1)).astype(np.float32)
    d["transflat"] = trans.reshape(K2, 1)

    dw = weights["dense_w"].astype(np.float32)                    # [HP, K]
    d["dwc"] = np.ascontiguousarray(
        dw.reshape(NKX, 128, Kn).transpose(1, 0, 2).reshape(128, NKX * Kn)
    ).astype(np.float16)
    d["db"] = (weights["dense_b"].reshape(1, Kn) - c0).astype(np.float32)
    d["ones1"] = np.ones((1, 128), np.float32)
    return d


def _get_runner(cfg):
    key = ("runner", cfg["T"], cfg["BL"], cfg["n_cores"])
    if key in _CACHE:
        return _CACHE[key]
    nc = build_nc(cfg)
    from concourse import bass2jax

    n_cores = cfg["n_cores"]

    import jax
    import numpy as _np
    from jax.sharding import Mesh, PartitionSpec
    from jax.experimental.shard_map import shard_map

    bass2jax.install_neuronx_cc_hook()
    partition_name = nc.partition_id_tensor.name if nc.partition_id_tensor else None
    import concourse.mybir as mybir

    in_names, out_names, out_avals = [], [], []
    for alloc in nc.m.functions[0].allocations:
        if not isinstance(alloc, mybir.MemoryLocationSet):
            continue
        name = alloc.memorylocations[0].name
        if alloc.kind == "ExternalInput":
            if name != partition_name:
                in_names.append(name)
        elif alloc.kind == "ExternalOutput":
            out_names.append(name)
            out_avals.append(
                jax.core.ShapedArray(tuple(alloc.tensor_shape), mybir.dt.np(alloc.dtype))
            )
    n_params = len(in_names)
    all_names = in_names + out_names
    if partition_name is not None:
        all_names = all_names + [partition_name]
    donate = tuple(range(n_params, n_params + len(out_names)))

    def _body(*args):
        operands = list(args)
        if partition_name is not None:
            operands.append(bass2jax.partition_id_tensor())
        outs = bass2jax._bass_exec_p.bind(
            *operands,
            out_avals=tuple(out_avals),
            in_names=tuple(all_names),
            out_names=tuple(out_names),
            lowering_input_output_aliases=(),
            sim_require_finite=True,
            sim_require_nnan=True,
            nc=nc,
        )
        return tuple(outs)

    devices = jax.devices()[:n_cores]

    class Runner:
        pass

    r = Runner()
    r.in_names, r.out_names, r.out_avals, r.n_cores = in_names, out_names, out_avals, n_cores
    if n_cores == 1:
        fn = jax.jit(_body, donate_argnums=donate, keep_unused=True)

        def pack(in_maps):
            return [np.asarray(in_maps[0][n]) for n in in_names]

        def call(packed):
            zeros = [np.zeros(a.shape, a.dtype) for a in out_avals]
            outs = fn(*packed, *zeros)
            return [{n: np.asarray(outs[i]) for i, n in enumerate(out_names)}]
    else:
        from jax.sharding import NamedSharding

        mesh = Mesh(_np.asarray(devices), ("core",))
        fn = jax.jit(
            shard_map(
                _body,
                mesh=mesh,
                in_specs=(PartitionSpec("core"),) * (n_params + len(out_names)),
                out_specs=(PartitionSpec("core"),) * len(out_names),
                check_rep=False,
            ),
            donate_argnums=donate,
            keep_unused=True,
        )
        sh = NamedSharding(mesh, PartitionSpec("core"))

        def pack(in_maps):
            concat_in = [
                np.concatenate([np.asarray(m[n]) for m in in_maps], axis=0)
                for n in in_names
            ]
            return [jax.device_put(a, sh) for a in concat_in]

        def call(packed):
            zeros = [
                np.zeros((n_cores * a.shape[0],) + tuple(a.shape[1:]), a.dtype)
                for a in out_avals
            ]
            outs = fn(*packed, *zeros)
            return [
                {
                    n: np.asarray(outs[i]).reshape((n_cores,) + tuple(out_avals[i].shape))[c]
                    for i, n in enumerate(out_names)
                }
                for c in range(n_cores)
            ]

    r.fn = fn
    r.pack = pack
    r.call = call

    def run(in_maps):
        return call(pack(in_maps))

    r.run = run
    _CACHE[key] = r
    return r


def make_in_maps(inputs, cfg):
    n_cores = cfg["n_cores"]
    BL = cfg["BL"]
    weights = {
        k: np.asarray(inputs[k], np.float32)
        for k in (
            "w_fw0", "b_fw0", "w_bw0", "b_bw0",
            "w_fw1", "b_fw1", "w_bw1", "b_bw1",
            "dense_w", "dense_b", "trans",
        )
    }
    emb = np.asarray(inputs["emb"], np.float32)
    lens = np.asarray(inputs["seq_lens"], np.int64)
    tgt = np.asarray(inputs["targets"], np.int64)
    in_maps = []
    for c in range(n_cores):
        sl = slice(c * BL, (c + 1) * BL)
        in_maps.append(_prep_core(emb[sl], lens[sl], tgt[sl], weights, cfg))
    return in_maps


def kernel(**inputs):
    cfg = dict(T=T, BL=B // N_CORES, E=E, H=H, K=K, n_cores=N_CORES)
    in_maps = make_in_maps(inputs, cfg)
    runner = _get_runner(cfg)
    res = runner.run(in_maps)
    total = sum(float(r["out"][0, 0]) for r in res)
    return np.asarray(np.float32(total / B))
